# revision 27
# baseline (speedup 1.0000x reference)
"""Trainium2 Bass kernel for EventDiffusion GNN (GCNConv + GATConv, 2 layers).

Sharding: nodes partitioned into 8 contiguous ranges (one per NeuronCore).
Each core aggregates messages for its destination-node range from replicated
tables (graph/data parallel per the sharding hint); the layer-1 output table
is exchanged with a chunked AllGather so every core can gather arbitrary
source rows for layer 2.

Dataflow per core (PSUM accumulation fp32):
  L1 (GCN):  gather X[src] rows per edge (bf16 512B rows = 1 DMA packet per
             row, the SWDGE sweet spot; HW dma_gather spread over 4 SWDGE
             queues), scatter-sum by dst-slot via host-precomputed one-hot
             matrices P1[e, j] = coeff_e * (dslot_e == j) streamed bf16:
             psum += P1^T @ gathered.  Then h = relu(AX @ W1 + b1) via PE
             transposes + matmuls, and the layer-2 row table
             t2[d, :] = [h@W2 | h.v1 | 1 | h.v2] is built locally per block.
  comm:      AllGather of the local t2 slice in 2 chunks, each issued as
             soon as its L1 blocks are done so the ring transfers hide
             under ongoing gather traffic.
  L2 (GAT):  2 passes split by source chunk; pass p's gathers read only
             collective chunk p (scoped AP => scoped dependency), so they
             start as soon as that ring lands.  Per-edge adst via
             psum_dv = PT^T @ adst (PT = transposed one-hot); alpha = exp
             of unshifted logits (logits are O(1), no overflow risk; the
             softmax shift cancels exactly in numerator/denominator);
             one-hot scaled by alpha on the Scalar engine;
             psum += (P2*alpha)^T @ gathered.  The softmax denominator
             rides along as the all-ones table column; pass 0 parks its
             partial in SBUF and the last pass normalizes + bias + relu.
"""

import numpy as np
import ml_dtypes

import concourse.bass as bass
import concourse.bacc as bacc
import concourse.mybir as mybir
import concourse.tile as tile
from concourse.bass_utils import run_bass_kernel_spmd

FP32 = mybir.dt.float32
BF16 = mybir.dt.bfloat16
I16 = mybir.dt.int16
NPBF16 = ml_dtypes.bfloat16

N_CORES = 8
D = 256
TCOLS = 384  # t2 row: [xw2(0:256) | asrc(256) | one(257) | adst(258) | pad)
TROW = TCOLS
NQ = 4       # SWDGE queues
CH = [4, 3, 3]   # L1 blocks per collective chunk == L2 pass
NCH = len(CH)


def _pad_nodes(n):
    return -(-n // (128 * N_CORES)) * (128 * N_CORES)


def _wrap16(idx):
    s = idx.astype(np.int16).reshape(-1, 16).T  # [16, L/16]
    return np.tile(s, (8, 1))  # [128, L/16]


# ----------------------------------------------------------------------------
# host-side preprocessing (graph structure only: indices + one-hot scatters)
# ----------------------------------------------------------------------------

def _tiles(src_l, dslot_l, coeff_l, Ts):
    """Pack per-(block) edge lists into 128-row tiles.

    Returns idxs [128, 8*st], P [128, st, 128] (one-hot, coeff-scaled when
    coeff is given) and PT [128, st, 128] (transposed plain one-hot).
    """
    st = sum(Ts)
    idxs = np.zeros((128, 8 * st), np.int16)
    P = np.zeros((128, st, 128), NPBF16)
    PT = np.zeros((128, st, 128), NPBF16)
    off = 0
    for s, j, co, T in zip(src_l, dslot_l, coeff_l, Ts):
        m = len(s)
        L = T * 128
        e = np.arange(m)
        t = off + e // 128
        p = e % 128
        if co is not None:
            P[p, t, j] = co.astype(NPBF16)
        else:
            P[p, t, j] = NPBF16(1.0)
        PT[j, t, p] = NPBF16(1.0)
        sfull = np.zeros(L, np.int64)
        sfull[:m] = s
        idxs[:, 8 * off: 8 * (off + T)] = _wrap16(sfull)
        off += T
    return idxs, P, PT


def _prep(event_emb, edge_index, W1, b1, W2, att_src, att_dst, b2):
    X = np.ascontiguousarray(np.asarray(event_emb, np.float32))
    n = X.shape[0]
    npad = _pad_nodes(n)
    per = npad // N_CORES
    nblk = per // 128

    ends = np.cumsum(CH)                 # chunk end blocks
    starts = ends - np.asarray(CH)       # chunk start blocks
    crows = [c * 128 for c in CH]        # rows per chunk per rank
    cbase = np.concatenate([[0], np.cumsum([N_CORES * r for r in crows])])

    ei = np.asarray(edge_index, np.int64)
    src = np.concatenate([ei[0], np.arange(n, dtype=np.int64)])
    dst = np.concatenate([ei[1], np.arange(n, dtype=np.int64)])
    deg = np.bincount(dst, minlength=n).astype(np.float32)
    dinv = np.where(deg > 0, 1.0 / np.sqrt(deg), 0.0).astype(np.float32)
    coeff = (dinv[src] * dinv[dst]).astype(np.float32)

    blk2ch = np.zeros(nblk, np.int64)
    for p in range(NCH):
        blk2ch[starts[p]:ends[p]] = p

    # chunk-major row permutation matching the on-device table layout
    # ([chunk][rank, row]): global row g -> cbase[ch] + c*crows[ch] + rr
    def _perm(g):
        c, r = g // per, g % per
        ch = blk2ch[r // 128]
        rr = r - starts[ch] * 128
        return cbase[ch] + c * np.asarray(crows)[ch] + rr

    key = (dst // per) * nblk + (dst % per) // 128
    order = np.argsort(key, kind="stable")
    src, dst, coeff, key = src[order], dst[order], coeff[order], key[order]
    srcp = _perm(src)
    schunk = blk2ch[(src % per) // 128]  # source collective chunk
    bounds = np.searchsorted(key, np.arange(N_CORES * nblk + 1))

    # per-(core, block) edge lists: full (L1) and split by source chunk (L2)
    s1 = [[] for _ in range(nblk)]
    j1 = [[] for _ in range(nblk)]
    c1 = [[] for _ in range(nblk)]
    sP = [[[] for _ in range(nblk)] for _ in range(NCH)]
    jP = [[[] for _ in range(nblk)] for _ in range(NCH)]
    for c in range(N_CORES):
        for b in range(nblk):
            lo, hi = bounds[c * nblk + b], bounds[c * nblk + b + 1]
            j = (dst[lo:hi] - (c * per + b * 128)).astype(np.int64)
            s1[b].append(srcp[lo:hi])
            j1[b].append(j)
            c1[b].append(coeff[lo:hi])
            ch = schunk[lo:hi]
            for p in range(NCH):
                m = ch == p
                # pass-p indices are rebased into the chunk-p sub-table so
                # the gather's input AP (= dependency) covers only chunk p
                sP[p][b].append(srcp[lo:hi][m] - cbase[p])
                jP[p][b].append(j[m])

    def tmax(ll):
        return [max(1, int(-(-max(len(x) for x in ll[b]) // 128)))
                for b in range(nblk)]

    T1 = tmax(s1)
    TP = [tmax(sP[p]) for p in range(NCH)]

    per_core = []
    for c in range(N_CORES):
        idxs1, p1, _ = _tiles(
            [s1[b][c] for b in range(nblk)], [j1[b][c] for b in range(nblk)],
            [c1[b][c] for b in range(nblk)], T1,
        )
        sl, jl, Tl = [], [], []
        for p in range(NCH):
            sl += [sP[p][b][c] for b in range(nblk)]
            jl += [jP[p][b][c] for b in range(nblk)]
            Tl += TP[p]
        idxs2, p2, pt = _tiles(sl, jl, [None] * len(sl), Tl)
        per_core.append(dict(idxs1=idxs1, p1=p1, idxs2=idxs2, p2=p2, pt=pt))

    W1 = np.asarray(W1, np.float32)
    W2 = np.asarray(W2, np.float32)
    v1 = W2 @ np.asarray(att_src, np.float32)
    v2 = W2 @ np.asarray(att_dst, np.float32)

    Xp = np.zeros((npad, D), NPBF16)
    Xp[_perm(np.arange(n))] = X.astype(NPBF16)

    ones384 = np.zeros((128, TCOLS), np.float32)
    ones384[:, 257] = 1.0

    W2p = np.zeros((D, TCOLS), np.float32)
    W2p[:, :D] = W2
    W2p[:, 256] = v1
    W2p[:, 258] = v2

    shared = dict(
        xtab=Xp,
        ones384=ones384,
        w1=np.ascontiguousarray(W1.reshape(2, 128, D).astype(NPBF16)),
        w2p=np.ascontiguousarray(W2p.reshape(2, 128, TCOLS).astype(NPBF16)),
        b1r=np.ascontiguousarray(
            np.asarray(b1, np.float32).reshape(2, 128, 1)
        ),
        b2b=np.ascontiguousarray(
            np.tile(np.asarray(b2, np.float32)[None, :], (128, 1))
        ),
        ident=np.eye(128, dtype=NPBF16),
    )
    return shared, per_core, (T1, TP), n, npad, per, nblk


# ----------------------------------------------------------------------------
# device program
# ----------------------------------------------------------------------------

def _build_nc(T1, TP, npad, per, nblk):
    st1 = sum(T1)
    Tl = [t for p in range(NCH) for t in TP[p]]
    st2 = sum(Tl)
    offs1 = np.concatenate([[0], np.cumsum(T1)]).astype(np.int64)
    offs2 = np.concatenate([[0], np.cumsum(Tl)]).astype(np.int64)
    ends = list(np.cumsum(CH))
    crows = [c * 128 for c in CH]
    nc = bacc.Bacc(
        "TRN2", target_bir_lowering=False, debug=False, num_devices=N_CORES,
        num_swdge_queues=NQ, dynamic_dma_scratch_size=32768,
    )

    # I/O
    xt_d = nc.dram_tensor("xtab", [npad, D], BF16, kind="ExternalInput")
    w1_d = nc.dram_tensor("w1", [2, 128, D], BF16, kind="ExternalInput")
    w2_d = nc.dram_tensor("w2p", [2, 128, TCOLS], BF16, kind="ExternalInput")
    b1_d = nc.dram_tensor("b1r", [2, 128, 1], FP32, kind="ExternalInput")
    b2_d = nc.dram_tensor("b2b", [128, D], FP32, kind="ExternalInput")
    ones_d = nc.dram_tensor("ones384", [128, TCOLS], FP32, kind="ExternalInput")
    ident_d = nc.dram_tensor("ident", [128, 128], BF16, kind="ExternalInput")
    idxs1_d = nc.dram_tensor("idxs1", [128, 8 * st1], I16, kind="ExternalInput")
    p1_d = nc.dram_tensor("p1", [128, st1, 128], BF16, kind="ExternalInput")
    idxs2_d = nc.dram_tensor("idxs2", [128, 8 * st2], I16, kind="ExternalInput")
    p2_d = nc.dram_tensor("p2", [128, st2, 128], BF16, kind="ExternalInput")
    pt_d = nc.dram_tensor("pt", [128, st2, 128], BF16, kind="ExternalInput")
    out_d = nc.dram_tensor("out_slice", [per, D], FP32, kind="ExternalOutput")

    # internal DRAM: per-chunk slice + allgathered table, chunk-major row
    # order matching the host permutation.  Separate tensors per chunk keep
    # each collective output contiguous and scope each pass's gather
    # dependency to its own chunk.
    t2s_d = [
        nc.dram_tensor(f"t2s{p}", [crows[p], TROW], BF16) for p in range(NCH)
    ]
    tabs = [
        nc.dram_tensor(
            f"table{p}", [N_CORES, crows[p], TROW], BF16, addr_space="Shared"
        )
        for p in range(NCH)
    ]
    tflat = [t.reshape([N_CORES * r, TROW]) for t, r in zip(tabs, crows)]

    mu, ad, mx = (
        mybir.AluOpType.mult,
        mybir.AluOpType.add,
        mybir.AluOpType.max,
    )
    qi = [0]

    def gather(g_sb, tab, idxs_sb, tb, o, ncols):
        """dma_gather of tb*128 rows into g_sb, ~1k rows per call, queues
        round-robin."""
        nch = NQ
        t0 = 0
        for ch in range(nch):
            t1 = min(tb, ((ch + 1) * tb + nch - 1) // nch)
            if t1 <= t0:
                continue
            nidx = (t1 - t0) * 128
            nc.gpsimd.dma_gather(
                g_sb[:, t0:t1, :],
                tab[:, :],
                idxs_sb[:, 8 * (o + t0): 8 * (o + t1)],
                num_idxs=nidx,
                num_idxs_reg=nidx,
                elem_size=ncols,
                single_packet=False,
                queue_num=qi[0],
            )
            qi[0] = (qi[0] + 1) % NQ
            t0 = t1

    def allgather(p):
        nc.gpsimd.collective_compute(
            "AllGather",
            mybir.AluOpType.bypass,
            replica_groups=[list(range(N_CORES))],
            ins=[t2s_d[p][:, :]],
            outs=[tabs[p][:, :, :]],
        )

    with tile.TileContext(nc) as tc:
        with tc.tile_pool(name="const", bufs=1) as cp:
            ident_sb = cp.tile([128, 128], BF16)
            nc.sync.dma_start(ident_sb[:], ident_d[:, :])
            b2_sb = cp.tile([128, D], FP32)
            nc.sync.dma_start(b2_sb[:], b2_d[:, :])
            ones_sb = cp.tile([128, TCOLS], FP32)
            nc.sync.dma_start(ones_sb[:], ones_d[:, :])
            b1_sb = cp.tile([128, 2, 1], FP32)
            w1_sb = cp.tile([128, 2, D], BF16)
            w2_sb = cp.tile([128, 2, TCOLS], BF16)
            for k in range(2):
                nc.sync.dma_start(w1_sb[:, k, :], w1_d[k])
                nc.sync.dma_start(w2_sb[:, k, :], w2_d[k])
                nc.sync.dma_start(b1_sb[:, k, :], b1_d[k])
            idxs1_sb = cp.tile([128, 8 * st1], I16)
            nc.sync.dma_start(idxs1_sb[:], idxs1_d[:, :])
            idxs2_sb = cp.tile([128, 8 * st2], I16)
            nc.sync.dma_start(idxs2_sb[:], idxs2_d[:, :])
            adst_sb = cp.tile([128, nblk], BF16)
            acc = cp.tile([128, nblk, 258], FP32)

            # ---------------- layer 1: GCN + local t2 slice ----------------
            with (
                tc.tile_pool(name="g1_p", bufs=3) as g1p,
                tc.tile_pool(name="p1_p", bufs=3) as p1p,
                tc.tile_pool(name="ax_p", bufs=2) as axp,
                tc.tile_pool(name="axt_p", bufs=2) as axtp,
                tc.tile_pool(name="ht_p", bufs=2) as htp,
                tc.tile_pool(name="t2_p", bufs=2) as t2p,
                tc.psum_pool(name="pax_p", bufs=3) as pax,
                tc.psum_pool(name="ptr_p", bufs=1) as ptr,
                tc.psum_pool(name="pht_p", bufs=2) as pht,
                tc.psum_pool(name="pt2_p", bufs=2) as pt2,
            ):
                chp = 0
                for b in range(nblk):
                    tb = T1[b]
                    o = int(offs1[b])
                    g1 = g1p.tile([128, tb, D], BF16, tag="g1")
                    gather(g1, xt_d, idxs1_sb, tb, o, D)
                    if chp < NCH - 1 and b == ends[chp] + 1:
                        allgather(chp)
                        chp += 1
                    p1t = p1p.tile([128, tb, 128], BF16, tag="p1")
                    nc.sync.dma_start(p1t[:], p1_d[:, o: o + tb, :])
                    ps = pax.tile([128, D], FP32, tag="ax")
                    for t in range(tb):
                        nc.tensor.matmul(
                            ps[:],
                            lhsT=p1t[:, t, :],
                            rhs=g1[:, t, :],
                            start=(t == 0),
                            stop=(t == tb - 1),
                        )
                    ax = axp.tile([128, D], BF16, tag="axs")
                    nc.vector.tensor_copy(ax[:], ps[:])
                    axt = axtp.tile([128, 2, 128], BF16, tag="axt")
                    for k in range(2):
                        ptt = ptr.tile([128, 128], BF16, tag="tr")
                        nc.tensor.transpose(
                            ptt[:], ax[:, k * 128:(k + 1) * 128], ident_sb[:]
                        )
                        nc.vector.tensor_copy(axt[:, k, :], ptt[:])
                    ht = htp.tile([128, 2, 128], BF16, tag="ht")
                    for fh in range(2):
                        ph = pht.tile([128, 128], FP32, tag="hT")
                        for k in range(2):
                            nc.tensor.matmul(
                                ph[:],
                                lhsT=w1_sb[:, k, fh * 128:(fh + 1) * 128],
                                rhs=axt[:, k, :],
                                start=(k == 0),
                                stop=(k == 1),
                            )
                        nc.scalar.activation(
                            ht[:, fh, :], ph[:],
                            mybir.ActivationFunctionType.Relu,
                            bias=b1_sb[:, fh, :],
                        )
                    p2b = pt2.tile([128, TCOLS], FP32, tag="t2")
                    for fh in range(2):
                        nc.tensor.matmul(
                            p2b[:],
                            lhsT=ht[:, fh, :],
                            rhs=w2_sb[:, fh, :],
                            start=(fh == 0),
                            stop=(fh == 1),
                        )
                    nc.vector.tensor_copy(adst_sb[:, b: b + 1], p2b[:, 258:259])
                    t2row = t2p.tile([128, TROW], BF16, tag="t2r")
                    nc.vector.tensor_tensor(
                        t2row[:], p2b[:], ones_sb[:], op=ad
                    )
                    wch = 0
                    while b >= ends[wch]:
                        wch += 1
                    r = b - (ends[wch - 1] if wch else 0)
                    nc.sync.dma_start(
                        t2s_d[wch][r * 128:(r + 1) * 128, :], t2row[:]
                    )
                allgather(NCH - 1)

            # ---------------- layer 2: GAT (NCH passes by src chunk) -------
            with (
                tc.tile_pool(name="g2_p", bufs=4) as g2p,
                tc.tile_pool(name="p2s_p", bufs=3) as p2p,
                tc.tile_pool(name="pts_p", bufs=3) as ptp,
                tc.tile_pool(name="sc_p", bufs=3) as scp,
                tc.tile_pool(name="m2_p", bufs=6) as m2p,
                tc.tile_pool(name="o_p", bufs=2) as op_,
                tc.psum_pool(name="pdv_p", bufs=2) as pdv,
                tc.psum_pool(name="pag_p", bufs=2) as pag,
            ):
                for p_ in range(NCH):
                    for b in range(nblk):
                        tb = TP[p_][b]
                        o = int(offs2[p_ * nblk + b])
                        g2 = g2p.tile([128, tb, TROW], BF16, tag="g2")
                        gather(g2, tflat[p_], idxs2_sb, tb, o, TROW)
                        p2t = p2p.tile([128, tb, 128], BF16, tag="p2")
                        nc.sync.dma_start(p2t[:], p2_d[:, o: o + tb, :])
                        ptt = ptp.tile([128, tb, 128], BF16, tag="pt")
                        nc.sync.dma_start(ptt[:], pt_d[:, o: o + tb, :])
                        # per-edge adst via transposed one-hot matmuls
                        dv = pdv.tile([128, tb], FP32, tag="dv")
                        for t in range(tb):
                            nc.tensor.matmul(
                                dv[:, t: t + 1],
                                lhsT=ptt[:, t, :],
                                rhs=adst_sb[:, b: b + 1],
                                start=True,
                                stop=True,
                            )
                        # alpha = exp(leaky_relu(asrc_src + adst_dst, 0.2))
                        t0 = scp.tile([128, tb], FP32, tag="t0")
                        nc.vector.tensor_tensor(
                            t0[:], g2[:, :, 256], dv[:], op=ad
                        )
                        e = scp.tile([128, tb], FP32, tag="e")
                        nc.vector.scalar_tensor_tensor(
                            e[:], t0[:], 0.2, t0[:], op0=mu, op1=mx
                        )
                        nc.vector.tensor_scalar_min(e[:], e[:], 60.0)
                        al = scp.tile([128, tb], FP32, tag="al")
                        nc.scalar.activation(
                            al[:], e[:], mybir.ActivationFunctionType.Exp
                        )
                        ps = pag.tile([128, 258], FP32, tag="agg")
                        for t in range(tb):
                            m2 = m2p.tile([128, 128], BF16, tag="m2")
                            nc.scalar.mul(m2[:], p2t[:, t, :], al[:, t: t + 1])
                            nc.tensor.matmul(
                                ps[:],
                                lhsT=m2[:],
                                rhs=g2[:, t, 0:258],
                                start=(t == 0),
                                stop=(t == tb - 1),
                            )
                        if p_ == 0:
                            nc.vector.tensor_copy(acc[:, b, :], ps[:])
                            continue
                        if p_ < NCH - 1:
                            nc.vector.tensor_tensor(
                                acc[:, b, :], ps[:], acc[:, b, :], op=ad
                            )
                            continue
                        num = op_.tile([128, 258], FP32, tag="num")
                        nc.vector.tensor_tensor(
                            num[:], ps[:], acc[:, b, :], op=ad
                        )
                        sden = scp.tile([128, 1], FP32, tag="sden")
                        nc.vector.tensor_scalar_add(
                            sden[:], num[:, 257:258], 1e-16
                        )
                        rc = scp.tile([128, 1], FP32, tag="rc")
                        nc.vector.reciprocal(rc[:], sden[:])
                        ob = op_.tile([128, D], FP32, tag="ob")
                        nc.vector.scalar_tensor_tensor(
                            ob[:], num[:, 0:D], rc[:], b2_sb[:], op0=mu, op1=ad
                        )
                        nc.vector.tensor_scalar_max(ob[:], ob[:], 0.0)
                        nc.sync.dma_start(
                            out_d[b * 128:(b + 1) * 128, :], ob[:]
                        )
    nc.finalize()
    return nc


# ----------------------------------------------------------------------------
# entry point
# ----------------------------------------------------------------------------

_CACHE = {}


def _get_nc(T1, TP, npad, per, nblk):
    key = (tuple(T1), tuple(tuple(t) for t in TP), npad, per, nblk)
    if key not in _CACHE:
        _CACHE[key] = _build_nc(T1, TP, npad, per, nblk)
    return _CACHE[key]


def kernel(event_emb, edge_index, W1, b1, W2, att_src, att_dst, b2,
           _want_results=False, _trace=False):
    shared, per_core, (T1, TP), n, npad, per, nblk = _prep(
        event_emb, edge_index, W1, b1, W2, att_src, att_dst, b2
    )
    nc = _get_nc(T1, TP, npad, per, nblk)
    in_maps = [{**shared, **per_core[c]} for c in range(N_CORES)]
    res = run_bass_kernel_spmd(
        nc, in_maps, core_ids=list(range(N_CORES)), trace=_trace
    )
    out = np.concatenate(
        [res.results[c]["out_slice"] for c in range(N_CORES)], axis=0
    )[:n]
    if _want_results:
        return out, res
    return out
